# revision 41
# baseline (speedup 1.0000x reference)
"""Trainium2 Bass kernel for nn_Attention_Layer (B=8, SH=SV=32, DH=D=256, DV=4096).

Math (see reference):
    U_h = h @ U                  (B,SH,D)
    W_v = v @ W                  (B,SV,D)
    f   = tanh(W_v + U_h + b)    (B,SH,SV,D)
    q   = f @ w                  (B,SH,SV,DV)
    e   = exp(q); S = sum_b e; beta = e/S
    u   = sum_sv beta * v        (B,SH,DV)

Sharding: the batch-axis normalization (sum over b) makes batch sharding need a
16MB all-reduce; sharding over SH instead keeps everything core-local.
Each of the 8 cores owns SH/8 = 4 h-positions, all batches. No collectives.

Per-core layouts:
  layout "fT":  f^T stored (d, (b,h,s)) so the q matmul uses f as the
                stationary operand and w as the moving operand.
  post-q "layout-1": partition=(h,s) [4*32=128], free=c' (DV), one tile per b.
                In this layout S=sum_b e is a PE identity-matmul accumulation,
                beta=e*R needs no broadcast, and u=sum_s (beta*v) is a PE
                matmul with a per-b block-indicator stationary matrix.
                v must be replicated across the 4 h-positions (SBUF->SBUF DMA).
"""

import sys

sys.path.insert(0, "/opt/trn_rl_repo")

from contextlib import ExitStack

import ml_dtypes
import numpy as np

import concourse.bass as bass
import concourse.mybir as mybir
import concourse.tile as tile
from concourse import bacc
from concourse.bass_utils import run_bass_kernel_spmd

BF16 = ml_dtypes.bfloat16
F32 = np.float32

B, SH, SV, DH, DV, D = 8, 32, 32, 256, 4096, 256
NCORES = 8
SHL = SH // NCORES  # 4 h-positions per core
ROWS = B * SHL  # 32 output rows per core, index = b*SHL + h
NT = 8  # number of c' tiles
NW = DV // NT  # 512 wide


def build_nc(debug: bool = False):
    nc = bacc.Bacc("TRN2", target_bir_lowering=False, debug=debug)
    f32, bf = mybir.dt.float32, mybir.dt.bfloat16

    W_d = nc.dram_tensor("W_bf", (DV, D), bf, kind="ExternalInput")
    vT_d = nc.dram_tensor("vT_bf", (DV, B * SV), bf, kind="ExternalInput")
    w_d = nc.dram_tensor("w_bf", (D, DV), bf, kind="ExternalInput")
    v3_d = nc.dram_tensor("v3_bf", (B * SV, DV), bf, kind="ExternalInput")
    U2_d = nc.dram_tensor("U2", (DH + 1, D), f32, kind="ExternalInput")
    hT2_d = nc.dram_tensor("hT2", (DH + 1, ROWS), f32, kind="ExternalInput")
    I_d = nc.dram_tensor("Ieye", (128, 128), bf, kind="ExternalInput")
    L_d = nc.dram_tensor("Lsum", (B, 128, ROWS), bf, kind="ExternalInput")
    u_d = nc.dram_tensor("u_out", (ROWS, DV), f32, kind="ExternalOutput")

    KT_C = DV // 128  # 32 k-tiles over the DV contraction (v @ W)
    KT_D = D // 128  # 2 k-tiles over the D contraction (f @ w)

    with tile.TileContext(nc) as tc, ExitStack() as ctx:
        consts = ctx.enter_context(tc.tile_pool(name="consts", bufs=1))

        ph1_ctx = ExitStack()
        ph1c = ph1_ctx.enter_context(tc.tile_pool(name="ph1c", bufs=1))

        # ---- resident constants -------------------------------------------
        # Sync queue carries HBM loads in dependency order (phase-1 weights
        # first); gpsimd queue carries the tiny consts + SBUF->SBUF v
        # replication so it runs in parallel without blocking a compute
        # engine's sequencer.
        U2_sb = ph1c.tile([128, 3, D], f32)
        nc.gpsimd.dma_start(out=U2_sb[:, 0, :], in_=U2_d[0:128, :])
        nc.gpsimd.dma_start(out=U2_sb[:, 1, :], in_=U2_d[128:256, :])
        nc.gpsimd.dma_start(out=U2_sb[0:1, 2, :], in_=U2_d[256:257, :])
        hT2_sb = ph1c.tile([128, 3, ROWS], f32)
        nc.gpsimd.dma_start(out=hT2_sb[:, 0, :], in_=hT2_d[0:128, :])
        nc.gpsimd.dma_start(out=hT2_sb[:, 1, :], in_=hT2_d[128:256, :])
        nc.gpsimd.dma_start(out=hT2_sb[0:1, 2, :], in_=hT2_d[256:257, :])

        # phase-1-only weights go in ph1c (freed before the big loop).
        # W/vT split into half-size chunks, interleaved so the Wv k-loop can
        # start after the first pair lands.
        W_hbm = W_d.rearrange("(kt p) d -> p kt d", p=128)
        vT_hbm = vT_d.rearrange("(kt p) s -> p kt s", p=128)
        W_sb = ph1c.tile([128, KT_C, D], bf)
        vT_sb = ph1c.tile([128, KT_C, B * SV], bf)
        for ck in range(0, KT_C, 8):
            cs = slice(ck, ck + 8)
            nc.sync.dma_start(out=W_sb[:, cs, :], in_=W_hbm[:, cs, :])
            nc.sync.dma_start(out=vT_sb[:, cs, :], in_=vT_hbm[:, cs, :])
        w_sb = consts.tile([128, KT_D, DV], bf)
        nc.sync.dma_start(out=w_sb, in_=w_d.rearrange("(kt p) c -> p kt c", p=128))
        I_sb = consts.tile([128, 128], bf)
        nc.gpsimd.dma_start(out=I_sb, in_=I_d[:])
        L_sb = consts.tile([128, B, ROWS], bf)
        nc.gpsimd.dma_start(out=L_sb, in_=L_d.rearrange("b p m -> p b m"))

        # v replicated over the 4 local h positions: partition (h,s), per b.
        v_rep = consts.tile([128, B, DV], bf)
        for bb in range(B):
            nc.sync.dma_start(
                out=v_rep[0:32, bb, :], in_=v3_d[bb * SV : (bb + 1) * SV, :]
            )
            for hh in range(1, SHL):
                nc.gpsimd.dma_start(
                    out=v_rep[hh * 32 : (hh + 1) * 32, bb, :],
                    in_=v_rep[0:32, bb, :],
                )

        # ---- phase 1: ubias = U^T h + bias, W_v^T, fT = tanh(...) ---------
        ph1 = ph1_ctx.enter_context(tc.tile_pool(name="ph1", bufs=1, space="PSUM"))

        ub_ps = ph1.tile([128, 2, ROWS], f32)
        for mt in range(2):
            msl = slice(mt * 128, (mt + 1) * 128)
            for kt in range(3):
                ksz = 128 if kt < 2 else 1
                nc.tensor.matmul(
                    ub_ps[:, mt, :],
                    U2_sb[0:ksz, kt, msl],
                    hT2_sb[0:ksz, kt, :],
                    start=(kt == 0),
                    stop=(kt == 2),
                )
        ub_sb = ph1c.tile([128, 2, ROWS], f32)
        nc.vector.tensor_copy(ub_sb, ub_ps)

        wv_ps = [
            ph1.tile([128, B * SV], f32, tag=f"wv{mt}", name=f"wv_ps{mt}")
            for mt in range(2)
        ]
        for kt in range(KT_C):  # kt-major so chunk 0 starts while chunk 1 loads
            for mt in range(2):
                msl = slice(mt * 128, (mt + 1) * 128)
                nc.tensor.matmul(
                    wv_ps[mt],
                    W_sb[:, kt, msl],
                    vT_sb[:, kt, :],
                    start=(kt == 0),
                    stop=(kt == KT_C - 1),
                )

        # zz[d, (b,h,s)] = W_v^T[d, (b,s)] + ubias[d, (b,h)], fT = tanh(zz)
        zz_sb = ph1c.tile([128, 2, B * SHL * SV], f32)
        fT_sb = consts.tile([128, KT_D, B * SHL * SV], bf)
        for mt in range(2):
            wv_base = wv_ps[mt][:]
            wv_bc = bass.AP(
                tensor=wv_base.tensor,
                offset=wv_base.offset,
                ap=[wv_base.ap[0], [32, B], [0, SHL], [1, SV]],
            )
            ub_base = ub_sb[:, mt, :]
            ub_bc = bass.AP(
                tensor=ub_base.tensor,
                offset=ub_base.offset,
                ap=[ub_base.ap[0], [SHL, B], [1, SHL], [0, SV]],
            )
            zz_out = zz_sb[:, mt, :].rearrange("p (b h s) -> p b h s", b=B, h=SHL)
            nc.vector.tensor_add(zz_out, wv_bc, ub_bc)
            for bh in range(2):  # split so the first q-matmuls start earlier
                bsl = slice(bh * 512, (bh + 1) * 512)
                nc.scalar.activation(
                    fT_sb[:, mt, bsl],
                    zz_sb[:, mt, bsl],
                    mybir.ActivationFunctionType.Tanh,
                )

        ph1_ctx.close()

        # ---- phase 2: q -> e -> S -> R -> beta*v -> u ---------------------
        # The u-matmul block for tile nt is emitted one iteration late so the
        # PE stream never waits on the DVE (g,gv) products of the same tile.
        epool = ctx.enter_context(tc.tile_pool(name="epool", bufs=20))
        gpool = ctx.enter_context(tc.tile_pool(name="gpool", bufs=4))
        gvpool = ctx.enter_context(tc.tile_pool(name="gvpool", bufs=18))
        usbpool = ctx.enter_context(tc.tile_pool(name="usbpool", bufs=3))
        r32pool = ctx.enter_context(tc.tile_pool(name="r32pool", bufs=3))
        rpool = ctx.enter_context(tc.tile_pool(name="rpool", bufs=3))
        qpool = ctx.enter_context(tc.tile_pool(name="qpool", bufs=2, space="PSUM"))
        spool = ctx.enter_context(tc.tile_pool(name="spool", bufs=2, space="PSUM"))
        upool = ctx.enter_context(tc.tile_pool(name="upool", bufs=2, space="PSUM"))

        pending = None  # (gv_tiles, nsl) for the deferred u-block

        NJ = 4  # col-group packing of the M=32 u-matmuls: 4 concurrent MMs
        NWJ = NW // NJ

        NPAIR = NT // 2  # q/exp run 1024-wide (two n-tiles at a time)
        NW2 = 2 * NW

        def emit_u_block(gv_tiles, nt):
            u_ps = upool.tile([ROWS, NW], f32)
            for bb in range(B):
                nc.tensor.matmul(
                    u_ps,
                    L_sb[:, bb, :],
                    gv_tiles[bb],
                    start=(bb == 0),
                    stop=(bb == B - 1),
                )
            u_sb = usbpool.tile([ROWS, NW], f32, tag="u_sb")
            nc.scalar.copy(u_sb, u_ps)
            nc.sync.dma_start(out=u_d[:, nt * NW : (nt + 1) * NW], in_=u_sb)

        for pr in range(NPAIR):
            e_pairs = [
                epool.tile([128, 2, NW], bf, tag="e", name=f"e_{pr}_{x}")
                for x in range(B)
            ]
            for bb in range(B):
                q_ps = qpool.tile([128, NW2], f32)
                for half in range(2):
                    nt = 2 * pr + half
                    nsl = slice(nt * NW, (nt + 1) * NW)
                    for kt in range(KT_D):
                        nc.tensor.matmul(
                            q_ps[:, half * NW : (half + 1) * NW],
                            fT_sb[:, kt, bb * 128 : (bb + 1) * 128],
                            w_sb[:, kt, nsl],
                            start=(kt == 0),
                            stop=(kt == KT_D - 1),
                            skip_group_check=True,
                        )
                nc.scalar.activation(
                    e_pairs[bb].rearrange("p a n -> p (a n)"),
                    q_ps,
                    mybir.ActivationFunctionType.Exp,
                )

            for half in range(2):
                nt = 2 * pr + half
                nsl = slice(nt * NW, (nt + 1) * NW)
                s_ps = spool.tile([128, NW], f32)
                for bb in range(B):
                    nc.tensor.matmul(
                        s_ps,
                        I_sb,
                        e_pairs[bb][:, half, :],
                        start=(bb == 0),
                        stop=(bb == B - 1),
                    )
                # R = 1/S: seed+Newton custom-DVE op (~18-bit), cast to bf16.
                s_sb = r32pool.tile([128, NW], f32, tag="s_sb")
                nc.vector.tensor_copy(s_sb, s_ps)
                r32 = r32pool.tile([128, NW], f32, tag="r32")
                nc.vector.reciprocal_approx_fast(r32, s_sb)
                r_sb = rpool.tile([128, NW], bf, tag="r")
                with nc.allow_low_precision(reason="bf16 weights in budget"):
                    nc.vector.tensor_copy(r_sb, r32)

                gv_tiles = []
                for bb in range(B):
                    g_t = gpool.tile([128, NW], bf, tag="g")
                    nc.vector.tensor_mul(g_t, e_pairs[bb][:, half, :], r_sb)
                    gv_t = gvpool.tile([128, NW], bf, tag="gv")
                    nc.vector.tensor_mul(gv_t, g_t, v_rep[:, bb, nsl])
                    gv_tiles.append(gv_t)

                if pending is not None:
                    emit_u_block(*pending)
                pending = (gv_tiles, nt)

        emit_u_block(*pending)

    nc.compile()
    return nc


def _install_profile_hook():
    """The image's antenv lacks axon_hooks; inject it and register the
    ctypes NTFF hook from trn_agent_boot so trace=True works under axon."""
    import types

    try:
        from antenv.axon_hooks import get_axon_ntff_profile_hook  # noqa: F401

        return
    except ImportError:
        pass
    import antenv

    mod = types.ModuleType("antenv.axon_hooks")
    holder = {"hook": None}
    mod.set_axon_ntff_profile_hook = lambda h: holder.__setitem__("hook", h)
    mod.get_axon_ntff_profile_hook = lambda: holder["hook"]
    sys.modules["antenv.axon_hooks"] = mod
    antenv.axon_hooks = mod
    try:
        if "/root/.axon_site" not in sys.path:
            sys.path.insert(0, "/root/.axon_site")
        from trn_agent_boot.trn_boot import _ntff_profile_via_ctypes

        mod.set_axon_ntff_profile_hook(
            _ntff_profile_via_ctypes("/opt/axon/libaxon_pjrt.so")
        )
    except Exception as ex:  # degrade: tracing skipped, run still works
        print("profile hook install failed:", ex)
    # artifact upload needs bucket creds this container doesn't have
    import concourse.bass_utils as bu

    bu.upload_artifacts = lambda tmpdir: "local://" + tmpdir


_NC_CACHE = {}


def _get_nc():
    if "nc" not in _NC_CACHE:
        _NC_CACHE["nc"] = build_nc()
    return _NC_CACHE["nc"]


def make_inputs(h, v, W, U, b, w):
    """Host-side prep: shared tensors + per-core in_maps."""
    W_bf = W.astype(BF16)
    vT_bf = np.ascontiguousarray(v.transpose(2, 0, 1).reshape(DV, B * SV)).astype(BF16)
    w_bf = w.astype(BF16)
    v3_bf = np.ascontiguousarray(v.reshape(B * SV, DV)).astype(BF16)
    U2 = np.concatenate([U, b[None, :]], axis=0).astype(F32)
    Ieye = np.eye(128, dtype=BF16)
    Lsum = np.zeros((B, 128, ROWS), dtype=BF16)
    for bb in range(B):
        for hh in range(SHL):
            for ss in range(SV):
                Lsum[bb, hh * SV + ss, bb * SHL + hh] = 1
    in_maps = []
    for core in range(NCORES):
        hsl = h[:, core * SHL : (core + 1) * SHL, :]  # (B, SHL, DH)
        hT = np.ascontiguousarray(hsl.transpose(2, 0, 1).reshape(DH, ROWS))
        hT2 = np.concatenate([hT, np.ones((1, ROWS), F32)], axis=0).astype(F32)
        in_maps.append(
            {
                "W_bf": W_bf,
                "vT_bf": vT_bf,
                "w_bf": w_bf,
                "v3_bf": v3_bf,
                "U2": U2,
                "hT2": hT2,
                "Ieye": Ieye,
                "Lsum": Lsum,
            }
        )
    return in_maps


def gather_output(results):
    u_full = np.empty((B, SH, DV), dtype=F32)
    for core, res in enumerate(results):
        u_full[:, core * SHL : (core + 1) * SHL, :] = res["u_out"].reshape(
            B, SHL, DV
        )
    return u_full


def kernel(h, v, W, U, b, w, trace: bool = False):
    if trace:
        _install_profile_hook()
    nc = _get_nc()
    in_maps = make_inputs(
        np.asarray(h, F32),
        np.asarray(v, F32),
        np.asarray(W, F32),
        np.asarray(U, F32),
        np.asarray(b, F32),
        np.asarray(w, F32),
    )
    out = run_bass_kernel_spmd(nc, in_maps, core_ids=list(range(NCORES)), trace=trace)
    res = gather_output(out.results)
    if trace:
        kernel.last_exec_time_ns = out.exec_time_ns
        kernel.last_trace = out.instructions_and_trace
    return res
